# revision 1
# baseline (speedup 1.0000x reference)
"""Trainium2 Bass kernel for BlockAttentionResidual.

Reference computation (fp32):
    K      = rmsnorm(V, w)                      # over d
    logits = einsum('d,lbtd->lbt', q, K)
    attn   = softmax(logits, axis=l)
    h      = einsum('lbt,lbtd->btd', attn, V)

Mapping (per NeuronCore, tokens = flattened (b,t) sharded 8 ways):
    logits[l,t] = inv[l,t] * dot(q*w, V[l,t,:]),  inv = rsqrt(mean(V^2)+eps)
    - dot      : DVE tensor_tensor_reduce (fused multiply+reduce, fp32)
    - sum(V^2) : ACT Square with accum_out
    - inv      : exp(-0.5*ln(mean+eps)) -- keeps every ACT function (Square,
                 Ln, Exp, Copy) inside the single `natural_log_exp_and_others`
                 table set, so no ACT table reloads
    - h        : 6 of 8 l-slices on PE as diag(attn_l) @ V_l accumulated in
                 PSUM (fp32 matmuls), last 2 on DVE scalar_tensor_tensor,
                 the first of which also merges PSUM -> SBUF
"""

from contextlib import ExitStack

import numpy as np

import concourse.bass as bass
import concourse.mybir as mybir
import concourse.tile as tile
from concourse import bacc
from concourse.bass_utils import run_bass_kernel_spmd

NCORES = 8
L = 8
B = 4
T = 4096
D = 1024
BT = B * T
TOK = BT // NCORES  # tokens per core
P = 128
NT = TOK // P  # token tiles per core
HALF = 512  # fp32 moving-operand limit per matmul
NPE = 6  # l-slices accumulated on the tensor engine
EPS = 1e-6
F32 = mybir.dt.float32

_CACHE: dict = {}

import os as _os

K_NT = int(_os.environ.get("K_NT", NT))
K_ACT_BCAST = int(_os.environ.get("K_ACT_BCAST", "1"))
K_NPE = int(_os.environ.get("K_NPE", NPE))
K_INPLACE = int(_os.environ.get("K_INPLACE", "1"))
# dot-product realization: "stt" (fused scalar_tensor_tensor + accum),
# "amr" (custom-DVE affine_mul_reduce), "classic" (mult + reduce split
# between DVE and ACT; K_RED_DVE of the 8 reduces stay on DVE)
K_DOT = _os.environ.get("K_DOT", "stt")
K_RED_DVE = int(_os.environ.get("K_RED_DVE", "4"))
K_DEEP = int(_os.environ.get("K_DEEP", "1"))


def _build_nc(nt=None, npe=None, dot=None, mode="full", reps=1, bigdma=False,
              dma_eng=None, diag_eng="dve", deep=None):
    deep = bool(K_DEEP) if deep is None else deep
    nt = K_NT if nt is None else nt
    npe = K_NPE if npe is None else npe
    dot = K_DOT if dot is None else dot
    A = mybir.ActivationFunctionType
    O = mybir.AluOpType
    X = mybir.AxisListType.X

    nc = bacc.Bacc(
        "TRN2",
        target_bir_lowering=False,
        debug=False,
        enable_asserts=False,
        num_devices=NCORES,
    )
    v_d = nc.dram_tensor("v", [L, TOK, D], F32, kind="ExternalInput")
    qwb_d = nc.dram_tensor("qwb", [P, D], F32, kind="ExternalInput")
    id_d = nc.dram_tensor("ident", [P, P], F32, kind="ExternalInput")
    h_d = nc.dram_tensor("h", [TOK, D], F32, kind="ExternalOutput")

    with tile.TileContext(nc) as tc, ExitStack() as ctx:
        deep = int(deep)
        vb, sb, db, hb, pb = {
            0: (2, 2, 3, 2, 2),
            1: (3, 3, 4, 3, 3),
            2: (4, 4, 6, 3, 4),
        }[min(deep, 2)]
        cpool = ctx.enter_context(tc.tile_pool(name="const", bufs=1))
        vpool = ctx.enter_context(tc.tile_pool(name="vin", bufs=vb))
        spool = ctx.enter_context(tc.tile_pool(name="small", bufs=sb))
        jpool = ctx.enter_context(tc.tile_pool(name="scratch", bufs=1))
        dpool = ctx.enter_context(tc.tile_pool(name="diag", bufs=db))
        hpool = ctx.enter_context(tc.tile_pool(name="hout", bufs=hb))
        ppool = ctx.enter_context(
            tc.tile_pool(name="psum", bufs=pb, space=bass.MemorySpace.PSUM)
        )

        qwb = cpool.tile([P, D], F32, tag="qwb")
        ident = cpool.tile([P, P], F32, tag="ident")
        nc.sync.dma_start(qwb[:], qwb_d[:])
        nc.sync.dma_start(ident[:], id_d[:])

        # stride-0 sinks for the full-size primary outputs of the fused
        # reduce ops (only the accum_out is consumed)
        if K_ACT_BCAST:
            jact = jpool.tile([P, 1], F32, tag="jact")
            jact_out = jact.broadcast_to((P, D))
        else:
            jact = jpool.tile([P, D], F32, tag="jact")
            jact_out = jact[:]
        jvec_bufs = 2 if dot == "classic" else 1

        # per-partition bias constants (no global const-AP registry here)
        zero_b = cpool.tile([P, 1], F32, tag="zero_b")
        eps_b = cpool.tile([P, 1], F32, tag="eps_b")
        nc.vector.memset(zero_b[:], 0.0)
        nc.vector.memset(eps_b[:], EPS)

        for rep_i in range(reps * nt):
            i = rep_i % nt
            if bigdma:
                vta = vpool.tile([P, L, D], F32, tag="vta", name="vta")
                nc.sync.dma_start(
                    vta[:],
                    v_d[:, i * P : (i + 1) * P, :].rearrange("l t d -> t l d"),
                )
                vt = [vta[:, l, :] for l in range(L)]
            else:
                vt = [
                    vpool.tile([P, D], F32, tag=f"v{l}", name=f"v{l}")
                    for l in range(L)
                ]
                eng = nc.sync if dma_eng is None else getattr(nc, dma_eng)
                for l in range(L):
                    eng.dma_start(vt[l][:], v_d[l, i * P : (i + 1) * P, :])

            if mode == "dmaonly":
                hs = hpool.tile([P, D], F32, tag="hs")
                nc.vector.tensor_copy(hs[:], vt[0][:])
                nc.sync.dma_start(h_d[i * P : (i + 1) * P, :], hs[:])
                continue
            ssq = spool.tile([P, L], F32, tag="ssq")
            dotv = spool.tile([P, L], F32, tag="dotv")
            if mode == "nodot":
                nc.vector.memset(dotv[:], 1.0)
            for l in range(L):
                nc.scalar.activation(
                    jact_out,
                    vt[l][:],
                    A.Square,
                    bias=zero_b[:],
                    accum_out=ssq[:, l : l + 1],
                )
                if mode == "nodot":
                    pass
                elif dot == "stt":
                    jvec = jpool.tile([P, D], F32, tag="jvec", bufs=1, name="jvec")
                    nc.vector.scalar_tensor_tensor(
                        jvec[:], vt[l][:], 1.0, qwb[:], O.mult, O.mult,
                        accum_out=dotv[:, l : l + 1],
                    )
                elif dot == "amr":
                    jvec = jpool.tile([P, D], F32, tag="jvec", bufs=1, name="jvec")
                    nc.vector.affine_mul_reduce(
                        jvec[:], dotv[:, l : l + 1], vt[l][:], qwb[:], 1.0, 0.0
                    )
                else:
                    # classic: DVE multiply, reduce split between DVE and ACT
                    jvec = jpool.tile(
                        [P, D], F32, tag="jvec", bufs=jvec_bufs, name="jvec"
                    )
                    nc.vector.tensor_mul(jvec[:], vt[l][:], qwb[:])
                    if l < K_RED_DVE:
                        nc.vector.tensor_reduce(
                            dotv[:, l : l + 1], jvec[:], X, O.add
                        )
                    else:
                        nc.scalar.activation(
                            jact_out, jvec[:], A.Copy,
                            accum_out=dotv[:, l : l + 1],
                        )

            # inv = rsqrt(mean + eps) = exp(-0.5 * ln(ssq/D + eps))
            lnm = spool.tile([P, L], F32, tag="lnm")
            nc.scalar.activation(lnm[:], ssq[:], A.Ln, scale=1.0 / D, bias=eps_b[:])
            inv = spool.tile([P, L], F32, tag="inv")
            nc.scalar.activation(inv[:], lnm[:], A.Exp, scale=-0.5, bias=zero_b[:])

            logits = spool.tile([P, L], F32, tag="logits")
            nc.vector.tensor_mul(logits[:], dotv[:], inv[:])
            nm = spool.tile([P, 1], F32, tag="nm")
            nc.vector.tensor_reduce(nm[:], logits[:], X, O.max, negate=True)
            e = spool.tile([P, L], F32, tag="e")
            s = spool.tile([P, 1], F32, tag="s")
            nc.scalar.activation(e[:], logits[:], A.Exp, bias=nm[:], accum_out=s[:])
            r = spool.tile([P, 1], F32, tag="r")
            nc.vector.reciprocal(r[:], s[:])
            fold_r = npe >= L and mode == "full"
            if fold_r:
                # unnormalized weights feed the diag matmuls; 1/s is applied
                # in the PSUM->SBUF copy below
                attn = e
            else:
                attn = spool.tile([P, L], F32, tag="attn")
                nc.vector.tensor_scalar_mul(attn[:], e[:], r[:])

            # h = sum_l attn_l * V_l : l < NPE via diag(attn_l) matmuls into
            # PSUM, remaining l on DVE
            if npe > 0:
                hp = ppool.tile([P, D], F32, tag="hp")
                for l in range(npe):
                    dg = dpool.tile([P, P], F32, tag="dg")
                    if diag_eng == "act":
                        nc.scalar.mul(dg[:], ident[:], attn[:, l : l + 1])
                    elif not fold_r:
                        # (ident * e_l) * r in one op -- diags don't wait on
                        # the attn tensor, only on e and r
                        nc.vector.tensor_scalar(
                            dg[:], ident[:], e[:, l : l + 1], r[:],
                            O.mult, O.mult,
                        )
                    else:
                        nc.vector.tensor_scalar_mul(
                            dg[:], ident[:], attn[:, l : l + 1]
                        )
                    for h_ in range(2):
                        nc.tensor.matmul(
                            hp[:, h_ * HALF : (h_ + 1) * HALF],
                            dg[:],
                            vt[l][:, h_ * HALF : (h_ + 1) * HALF],
                            start=(l == 0),
                            stop=(l == npe - 1),
                        )
                hs = hpool.tile([P, D], F32, tag="hs")
                if npe >= L:
                    if fold_r:
                        nc.scalar.mul(hs[:], hp[:], r[:])
                    else:
                        nc.scalar.copy(hs[:], hp[:])
                    rest = range(L, L)
                else:
                    nc.vector.scalar_tensor_tensor(
                        hs[:], vt[npe][:], attn[:, npe : npe + 1], hp[:],
                        O.mult, O.add,
                    )
                    rest = range(npe + 1, L)
            else:
                hs = hpool.tile([P, D], F32, tag="hs")
                nc.vector.tensor_scalar_mul(hs[:], vt[0][:], attn[:, 0:1])
                rest = range(1, L)
            for l in rest:
                if K_INPLACE:
                    nc.vector.scalar_tensor_tensor(
                        hs[:], vt[l][:], attn[:, l : l + 1], hs[:], O.mult, O.add
                    )
                else:
                    hs2 = hpool.tile([P, D], F32, tag="hs", name="hs2")
                    nc.vector.scalar_tensor_tensor(
                        hs2[:], vt[l][:], attn[:, l : l + 1], hs[:], O.mult, O.add
                    )
                    hs = hs2
            nc.sync.dma_start(h_d[i * P : (i + 1) * P, :], hs[:])

    nc.compile()
    return nc


def get_nc():
    if "nc" not in _CACHE:
        _CACHE["nc"] = _build_nc()
    return _CACHE["nc"]


def build_variant(**kw):
    return _build_nc(**kw)


def make_in_maps(blocks, query, norm_weight):
    qw = (query * norm_weight).astype(np.float32)
    qwb = np.ascontiguousarray(np.broadcast_to(qw, (P, D)))
    ident = np.eye(P, dtype=np.float32)
    vr = blocks.reshape(L, BT, D)
    return [
        {
            "v": np.ascontiguousarray(vr[:, c * TOK : (c + 1) * TOK, :]),
            "qwb": qwb,
            "ident": ident,
        }
        for c in range(NCORES)
    ]


def kernel(blocks, query, norm_weight):
    import time

    blocks = np.asarray(blocks, dtype=np.float32)
    query = np.asarray(query, dtype=np.float32)
    norm_weight = np.asarray(norm_weight, dtype=np.float32)
    nc = get_nc()
    in_maps = make_in_maps(blocks, query, norm_weight)
    last_exc = None
    for attempt in range(3):
        try:
            res = run_bass_kernel_spmd(nc, in_maps, core_ids=list(range(NCORES)))
            break
        except Exception as exc:  # transient device-wedge after a prior crash
            last_exc = exc
            time.sleep(45)
    else:
        raise last_exc
    h = np.concatenate([res.results[c]["h"] for c in range(NCORES)], axis=0)
    return h.reshape(B, T, D)



# revision 4
# speedup vs baseline: 171.0187x; 171.0187x over previous
"""Trainium2 Bass kernel for BlockAttentionResidual (bf16 traffic).

    K      = rmsnorm(V, w);  logits = q @ K;  attn = softmax_l(logits)
    h      = sum_l attn_l * V_l

Design (vs the fp32 l-major baseline, ~3.9x faster):
  - Host pre-transposes each core's shard to token-major [TOK, L, D]
    and casts to bf16; h returns bf16 and is upcast on the host.
    Halves HBM traffic (32MB in + 2MB out per core); measured
    end-to-end rms-rel error 8.2e-3 against the fp32 jax reference
    (gate 2e-2). One contiguous 2MB dma_start per 128-token tile.
  - All ACT functions (Square, Ln, Exp, Copy) pinned to the single
    `natural_log_exp_and_others` table set; bass's default chooser
    flip-flopped exp->`exp_and_others` / ln->`natural_log`, inserting
    2 ACT table reloads (~2.6us) per tile = 42us/pass.
  - attn diag matrices built with single-scalar tensor_scalar_mul in
    bf16 (fast DVE path).
  - Output store issued from ACT right after its PSUM->SBUF copy of
    the PREVIOUS tile (software-pipelined), so stores never
    head-of-line block the input loads on SP's HWDGE ring.

Engine split per tile (tokens on partitions; bf16 doubles DVE rates):
    ACT    : n_act_sq squares (ssq accum), Ln, Exp (inv), Exp+accum
             (e,s), PSUM->SBUF bf16 copy + output dma_start for the
             PREVIOUS tile (software-pipelined)
    DVE    : 8 fused dots + (8 - n_act_sq) squares (STT, bf16 2x),
             attn=e*r (bf16), 8 diag builds (bf16 4x), logits mul,
             max-reduce, recip
    PE     : 8 diag(attn_l) @ V_l bf16 matmuls accumulated in fp32 PSUM
    SP     : input dma_start only
"""

import types
from contextlib import ExitStack

import numpy as np
import ml_dtypes

import bass_rust as _bass_rust
import concourse.bass as bass
import concourse.hw_specs as hw_specs
import concourse.mybir as mybir
import concourse.tile as tile
from concourse import bacc
from concourse.bass_utils import run_bass_kernel_spmd

ACT_SET = "natural_log_exp_and_others"


def _pin_act_tables(nc):
    """Force every ACT function onto one table set so no reloads occur.
    Set ids are positional (index into act_info.json), so we keep the
    list shape and empty out every other set's function list."""

    def patched(self):
        has_activation = any(
            isinstance(i, mybir.InstActivation)
            for b in self.main_func.blocks
            for i in b.instructions
        )
        if not has_activation:
            return
        tables = [
            (name, (fns if name == ACT_SET else set()))
            for name, fns in hw_specs.get_activation_tables(self.m.arch).items()
        ]
        _bass_rust.insert_act_table_loads(self, tables)

    nc.insert_act_table_loads = types.MethodType(patched, nc)

NCORES = 8
L = 8
B = 4
T = 4096
D = 1024
BT = B * T
TOK = BT // NCORES  # tokens per core
P = 128
NT = TOK // P  # token tiles per core
HALF = 512  # PSUM-bank limit on matmul free dim
EPS = 1e-6
F32 = mybir.dt.float32
BF16 = mybir.dt.bfloat16

_CACHE: dict = {}


def _build_nc(reps=1, vb=6, hb=3, pb=3, n_act_sq=7, copy_eng="scalar",
              act_real=1, mode="full", loop=None):
    A = mybir.ActivationFunctionType
    O = mybir.AluOpType
    X = mybir.AxisListType.X

    nc = bacc.Bacc(
        "TRN2",
        target_bir_lowering=False,
        debug=False,
        enable_asserts=False,
        num_devices=NCORES,
    )
    _pin_act_tables(nc)
    v_d = nc.dram_tensor("v", [TOK, L, D], BF16, kind="ExternalInput")
    qwb_d = nc.dram_tensor("qwb", [P, D], BF16, kind="ExternalInput")
    id_d = nc.dram_tensor("ident", [P, P], BF16, kind="ExternalInput")
    h_d = nc.dram_tensor("h", [TOK, D], BF16, kind="ExternalOutput")

    with tile.TileContext(nc) as tc, ExitStack() as ctx:
        cpool = ctx.enter_context(tc.tile_pool(name="const", bufs=1))
        vpool = ctx.enter_context(tc.tile_pool(name="vin", bufs=vb))
        spool = ctx.enter_context(tc.tile_pool(name="small", bufs=4))
        jpool = ctx.enter_context(tc.tile_pool(name="scratch", bufs=1))
        dpool = ctx.enter_context(tc.tile_pool(name="diag", bufs=4))
        hpool = ctx.enter_context(tc.tile_pool(name="hout", bufs=hb))
        ppool = ctx.enter_context(
            tc.tile_pool(name="psum", bufs=pb, space=bass.MemorySpace.PSUM)
        )

        qwb = cpool.tile([P, D], BF16, tag="qwb")
        ident = cpool.tile([P, P], BF16, tag="ident")
        nc.sync.dma_start(qwb[:], qwb_d[:])
        nc.sync.dma_start(ident[:], id_d[:])

        # stride-0 sink for ACT Square primary output (only accum consumed)
        jact = jpool.tile([P, 1], F32, tag="jact")
        jact_out = jact.broadcast_to((P, D))
        # dense bf16 sink (sqreal mode / act_real option)
        jreal = jpool.tile([P, D], BF16, tag="jreal")
        # full-size scratch for the fused-dot primary outputs (DVE-only)
        jvec_v = jpool.tile([P, D], BF16, tag="jvec_v")

        zero_b = cpool.tile([P, 1], F32, tag="zero_b")
        eps_b = cpool.tile([P, 1], F32, tag="eps_b")
        nc.vector.memset(zero_b[:], 0.0)
        nc.vector.memset(eps_b[:], EPS)

        def flush_prev(prev):
            if prev is None:
                return
            hp_p, i_p = prev
            hs = hpool.tile([P, D], BF16, tag="hs")
            if copy_eng == "scalar":
                nc.scalar.copy(hs[:], hp_p[:])
            else:
                nc.vector.tensor_copy(hs[:], hp_p[:])
            nc.scalar.dma_start(h_d[i_p * P : (i_p + 1) * P, :], hs[:])

        def emit_tile(i, prev):
            vta = vpool.tile([P, L, D], BF16, tag="vta", name="vta")
            nc.sync.dma_start(vta[:], v_d[i * P : (i + 1) * P, :, :])
            vt = [vta[:, l, :] for l in range(L)]

            if mode == "dmaonly":
                hs = hpool.tile([P, D], BF16, tag="hs")
                nc.vector.tensor_copy(hs[:], vt[0])
                nc.scalar.dma_start(h_d[i * P : (i + 1) * P, :], hs[:])
                return None

            if mode == "dotonly":
                # isolate the 8 DVE fused-dot scans (ACT does the token copy)
                dotv = spool.tile([P, L], F32, tag="dotv")
                for l in range(L):
                    nc.vector.scalar_tensor_tensor(
                        jvec_v[:], vt[l], 1.0, qwb[:], O.mult, O.mult,
                        accum_out=dotv[:, l : l + 1],
                    )
                hs = hpool.tile([P, D], BF16, tag="hs")
                nc.scalar.copy(hs[:], vt[0])
                nc.scalar.dma_start(h_d[i * P : (i + 1) * P, :], hs[:])
                return None

            if mode == "sqonly":
                # isolate the 8 ACT square scans (DVE does the token copy)
                ssq = spool.tile([P, L], F32, tag="ssq")
                for l in range(L):
                    nc.scalar.activation(
                        jact_out, vt[l], A.Square,
                        bias=zero_b[:], accum_out=ssq[:, l : l + 1],
                    )
                hs = hpool.tile([P, D], BF16, tag="hs")
                nc.vector.tensor_copy(hs[:], vt[0])
                nc.scalar.dma_start(h_d[i * P : (i + 1) * P, :], hs[:])
                return None

            if mode == "sqreal":
                # ACT squares writing a REAL dense bf16 tile (tests whether
                # the stride-0 broadcast sink blocks a 2x out-side rate)
                ssq = spool.tile([P, L], F32, tag="ssq")
                for l in range(L):
                    nc.scalar.activation(
                        jreal[:], vt[l], A.Square,
                        bias=zero_b[:], accum_out=ssq[:, l : l + 1],
                    )
                hs = hpool.tile([P, D], BF16, tag="hs")
                nc.vector.tensor_copy(hs[:], vt[0])
                nc.scalar.dma_start(h_d[i * P : (i + 1) * P, :], hs[:])
                return None

            ssq = spool.tile([P, L], F32, tag="ssq")
            dotv = spool.tile([P, L], F32, tag="dotv")
            sq_out = jreal[:] if act_real else jact_out
            for l in range(L):
                if l < n_act_sq:
                    nc.scalar.activation(
                        sq_out, vt[l], A.Square,
                        bias=zero_b[:], accum_out=ssq[:, l : l + 1],
                    )
                else:
                    nc.vector.scalar_tensor_tensor(
                        jvec_v[:], vt[l], 1.0, vt[l], O.mult, O.mult,
                        accum_out=ssq[:, l : l + 1],
                    )
                nc.vector.scalar_tensor_tensor(
                    jvec_v[:], vt[l], 1.0, qwb[:], O.mult, O.mult,
                    accum_out=dotv[:, l : l + 1],
                )

            # inv = rsqrt(mean + eps) = exp(-0.5 * ln(ssq/D + eps))
            lnm = spool.tile([P, L], F32, tag="lnm")
            nc.scalar.activation(lnm[:], ssq[:], A.Ln, scale=1.0 / D, bias=eps_b[:])
            inv = spool.tile([P, L], F32, tag="inv")
            nc.scalar.activation(inv[:], lnm[:], A.Exp, scale=-0.5, bias=zero_b[:])

            # previous tile's PSUM -> SBUF + store fills the gap on ACT
            flush_prev(prev)

            logits = spool.tile([P, L], F32, tag="logits")
            nc.vector.tensor_mul(logits[:], dotv[:], inv[:])
            nm = spool.tile([P, 1], F32, tag="nm")
            nc.vector.tensor_reduce(nm[:], logits[:], X, O.max, negate=True)
            e = spool.tile([P, L], F32, tag="e")
            s = spool.tile([P, 1], F32, tag="s")
            nc.scalar.activation(e[:], logits[:], A.Exp, bias=nm[:], accum_out=s[:])
            r = spool.tile([P, 1], F32, tag="r")
            nc.vector.reciprocal(r[:], s[:])
            attn = spool.tile([P, L], F32, tag="attn")
            nc.vector.tensor_scalar_mul(attn[:], e[:], r[:])

            # h = sum_l attn_l * V_l via diag(attn_l) bf16 matmuls in PSUM
            hp = ppool.tile([P, D], F32, tag="hp")
            for l in range(L):
                dg = dpool.tile([P, P], BF16, tag="dg")
                nc.vector.tensor_scalar_mul(dg[:], ident[:], attn[:, l : l + 1])
                for h_ in range(2):
                    nc.tensor.matmul(
                        hp[:, h_ * HALF : (h_ + 1) * HALF],
                        dg[:],
                        vt[l][:, h_ * HALF : (h_ + 1) * HALF],
                        start=(l == 0),
                        stop=(l == L - 1),
                    )
            return (hp, i)

        if loop is not None:
            with tc.For_i(0, loop, 1):
                prev = None
                for i in range(NT):
                    prev = emit_tile(i, prev)
                flush_prev(prev)
        else:
            prev = None
            for rep_i in range(reps * NT):
                prev = emit_tile(rep_i % NT, prev)
            flush_prev(prev)

    nc.compile()
    return nc


def get_nc():
    if "nc" not in _CACHE:
        _CACHE["nc"] = _build_nc()
    return _CACHE["nc"]


def build_variant(**kw):
    return _build_nc(**kw)


def make_in_maps(blocks, query, norm_weight):
    qw = (query * norm_weight).astype(ml_dtypes.bfloat16)
    qwb = np.ascontiguousarray(np.broadcast_to(qw, (P, D)))
    ident = np.eye(P, dtype=ml_dtypes.bfloat16)
    vr = blocks.reshape(L, BT, D)
    return [
        {
            # [L, TOK, D] -> token-major bf16 [TOK, L, D]
            "v": np.ascontiguousarray(
                vr[:, c * TOK : (c + 1) * TOK, :].transpose(1, 0, 2)
            ).astype(ml_dtypes.bfloat16),
            "qwb": qwb,
            "ident": ident,
        }
        for c in range(NCORES)
    ]


def kernel(blocks, query, norm_weight):
    import time

    blocks = np.asarray(blocks, dtype=np.float32)
    query = np.asarray(query, dtype=np.float32)
    norm_weight = np.asarray(norm_weight, dtype=np.float32)
    nc = get_nc()
    in_maps = make_in_maps(blocks, query, norm_weight)
    last_exc = None
    for attempt in range(3):
        try:
            res = run_bass_kernel_spmd(nc, in_maps, core_ids=list(range(NCORES)))
            break
        except Exception as exc:  # transient device-wedge after a prior crash
            last_exc = exc
            time.sleep(45)
    else:
        raise last_exc
    h = np.concatenate([res.results[c]["h"] for c in range(NCORES)], axis=0)
    return h.astype(np.float32).reshape(B, T, D)
